# revision 1
# baseline (speedup 1.0000x reference)
"""Trainium2 Bass kernel for nn_DataReuploadingTorso (8-qubit data-reuploading
quantum circuit, batch 16384).

Math: each PennyLane Rot(phi,theta,omega) = RZ(omega+pi/2) H RZ(theta) H RZ(phi-pi/2).
The circuit becomes 17 diagonal phase layers interleaved with H^{x8} transforms,
plus |.|^2 and a fixed Z-projection. Four of the 17 diagonals (the inner
RZ(theta) of each of the 4 layers) are batch-independent: their cos/sin vectors
are precomputed on host and DMA'd in, skipping the phase matmul + sin/cos
pipeline entirely. The final RZ of the last Rot is dropped (|.|^2 invariant).

H^{x8} = H_1 (x) H^{x7}: apply M = H^{x7}/16 as a 128-contract matmul on the
low 7 state bits (s_low = partitions), then the top-bit H_1 as a 2-element
butterfly across free-dim halves (2 DVE adds/subs per component) - this halves
the TensorE instruction count vs the block [[M,M],[M,-M]] form.

Phases: q_k = Phi_k/(2*pi) comes from one 13-contract matmul per step
(x_aug -> all 256 states, both halves packed [h x batch] in free dim).
sin/cos: f = q - rne(q) via DVE f32->i32 convert (round-to-nearest) reading
PSUM directly, then ACT Sin(2*pi*f); cos path uses rne(q+0.25) so no ACT bias
is needed.

Layout: one pass over the core's full batch (2048): every elementwise
instruction is [128, 4096] wide. Per-instruction overheads dominate on this
target, so the kernel minimizes instruction count (~0.65k vs ~4.8k baseline).

Sharding: pure data-parallel over batch across 8 NeuronCores (2048 each).
"""
import os

import numpy as np

import concourse.bass as bass
import concourse.mybir as mybir
import concourse.tile as tile
from concourse.bass import ds
from concourse.bass_utils import run_bass_kernel_spmd

N_CORES = 8
B_TOTAL = 16384
B_CORE = B_TOTAL // N_CORES      # 2048
W2 = 2 * B_CORE                  # 4096 free-dim cols: [h=0 batch | h=1 batch]
NSTEP = 17
DIM = 256
N_Q = 8

F32 = mybir.dt.float32
F32R = mybir.dt.float32r
F16 = mybir.dt.float16
I32 = mybir.dt.int32
I16 = mybir.dt.int16
AOT = mybir.AluOpType
ACTF = mybir.ActivationFunctionType

TWO_PI = float(np.float32(2.0 * np.pi))

KREP = int(os.environ.get("KREP", "1"))  # debug: repeat body for timing probes

# step indices (0-based) whose phase is batch-independent (A == 0): the
# theta[:,l,1] inner-RZ of each layer
BI_STEPS = (2, 6, 10, 14)
BD_STEPS = tuple(k for k in range(NSTEP) if k not in BI_STEPS)  # 13 steps
BD_POS = {k: i for i, k in enumerate(BD_STEPS)}  # -> slice of packed W


# ----------------------------------------------------------------- host tables
def _build_host_tables(theta, omega):
    """W (13, 13*256) phase weights for the 13 batch-dep steps; CS (4, 2, 128,
    W2) broadcast cos/sin for the 4 batch-indep steps; M (128,128) = H^{x7}/16;
    Z (128, 16) = [Z_h0 | Z_h1] PauliZ stationaries."""
    theta = np.asarray(theta, np.float64)              # (8, 5, 3)
    omega = np.asarray(omega, np.float64).reshape(5, 8, 3)

    idx = np.arange(DIM)
    beta = np.stack([(idx >> (7 - q)) & 1 for q in range(N_Q)], 0)   # (8, 256)
    sgn = (2 * beta - 1).astype(np.float64)

    def czterm(pairs):
        t = np.zeros(DIM)
        for a, b in pairs:
            t += np.pi * (beta[a] * beta[b])
        return t
    cz_even = czterm([(0, 1), (2, 3), (4, 5), (6, 7)])
    cz_odd = czterm([(1, 2), (3, 4), (5, 6)])

    steps = []
    for l in range(4):
        A = np.zeros((12, 8)); k = np.zeros(8)
        for q in range(8):
            A[3 * (q % 4) + 1, q] = omega[l, q, 1]
        steps.append((A, k, None))
        A = np.zeros((12, 8)); k = np.zeros(8)
        for q in range(8):
            A[3 * (q % 4) + 2, q] = omega[l, q, 2]
            k[q] = theta[q, l, 0]
        steps.append((A, k, None))
        A = np.zeros((12, 8)); k = theta[:, l, 1].copy()
        steps.append((A, k, None))                     # batch-independent
        A = np.zeros((12, 8)); k = theta[:, l, 2].copy()
        for q in range(8):
            A[3 * (q % 4) + 0, q] = omega[l + 1, q, 0]
        if l + 1 == 4:
            k += theta[:, 4, 0]
        steps.append((A, k, cz_even if l % 2 == 0 else cz_odd))
    A = np.zeros((12, 8)); k = theta[:, 4, 1].copy()
    for q in range(8):
        A[3 * (q % 4) + 1, q] = omega[4, q, 1]
    steps.append((A, k, None))

    inv2pi = 1.0 / (2.0 * np.pi)
    W = np.zeros((13, len(BD_STEPS) * DIM))
    # CS[p, 4j + h] = cos(phi_j[h*128 + p]); CS[p, 4j + 2 + h] = sin(...)
    CS = np.zeros((128, 4 * len(BI_STEPS)), np.float32)
    for i, (A, k, cz) in enumerate(steps):
        c = k @ (sgn * 0.5)
        if cz is not None:
            c = c + cz
        if i in BI_STEPS:
            j = BI_STEPS.index(i)
            for h in (0, 1):
                CS[:, 4 * j + 0 + h] = np.cos(c[h * 128:(h + 1) * 128])
                CS[:, 4 * j + 2 + h] = np.sin(c[h * 128:(h + 1) * 128])
        else:
            base = BD_POS[i] * DIM
            W[:12, base:base + DIM] = (A @ (sgn * 0.5)) * inv2pi
            W[12, base:base + DIM] = \
                (np.mod(c + np.pi, 2 * np.pi) - np.pi) * inv2pi

    sp = np.arange(128)
    pop = np.zeros((128, 128), np.int64)
    for q in range(7):
        pop += np.outer((sp >> q) & 1, (sp >> q) & 1)
    M = (np.where(pop % 2 == 0, 1.0, -1.0) / 16.0).astype(np.float32)

    Z = np.stack([1.0 - 2.0 * ((idx >> (7 - q)) & 1) for q in range(8)], 1)
    Zp = np.concatenate([Z[:128], Z[128:]], 1).astype(np.float32)  # (128, 16)
    return W.astype(np.float32), CS, M, Zp


# -------------------------------------------------------------- device program
def _legalize_waits(nc, limit=1, limit_other=None):
    """walrus codegen allows only one embedded sync-wait on several TRN2
    instruction encodings (notably self-loading fp32/fp32r Matmult). Hoist
    excess waits emitted by Tile onto same-engine NoOps placed just before."""
    if limit_other is None:
        limit_other = limit
    one_wait = (mybir.InstMatmult, mybir.InstTensorScalarPtr)

    def fix_block(blk):
        new_insts = []
        for ins in blk.instructions:
            lim = limit if isinstance(ins, one_wait) else limit_other
            si = getattr(ins, "sync_info", None)
            waits = list(si.on_wait) if si and si.on_wait else []
            if len(waits) > lim:
                keep = waits[-lim:]
                for j, w in enumerate(waits[:-lim]):
                    new_insts.append(mybir.InstNoOp(
                        name=f"{ins.name}-w{j}",
                        engine=ins.engine,
                        sync_info=mybir.SyncInfo(on_wait=[w], on_update=[]),
                    ))
                si.on_wait = keep
            new_insts.append(ins)
        blk.instructions = new_insts
        for sb in getattr(blk, "blocks", None) or []:
            fix_block(sb)
    for f in nc.m.functions:
        for blk in f.blocks:
            fix_block(blk)


def _build_program():
    nc = bass.Bass("TRN2", target_bir_lowering=False, debug=False,
                   enable_asserts=False, num_devices=N_CORES)

    xT_d = nc.dram_tensor("xT", [13, B_CORE], F32, kind="ExternalInput")
    W_d = nc.dram_tensor("W", [13, len(BD_STEPS) * DIM], F32,
                         kind="ExternalInput")
    M_d = nc.dram_tensor("M", [128, 128], F32, kind="ExternalInput")
    Z_d = nc.dram_tensor("Zt", [128, 16], F32, kind="ExternalInput")
    CS_d = nc.dram_tensor("CS", [128, 4 * len(BI_STEPS)], F32,
                          kind="ExternalInput")
    out_d = nc.dram_tensor("out", [8, B_CORE], F32, kind="ExternalOutput")

    with tile.TileContext(nc) as tc:
        with (
            tc.tile_pool(name="consts", bufs=1) as consts,
            tc.tile_pool(name="psum", bufs=1, space="PSUM") as psum_pool,
            tc.tile_pool(name="iq", bufs=1) as iq_pool,
            tc.tile_pool(name="fr", bufs=1) as f_pool,
            tc.tile_pool(name="sc", bufs=2) as sc_pool,
            tc.tile_pool(name="pp", bufs=2) as pp_pool,
            tc.tile_pool(name="prod", bufs=2) as prod_pool,
            tc.tile_pool(name="st", bufs=2) as st_pool,
            tc.tile_pool(name="tail", bufs=1) as tail_pool,
        ):
            xT = consts.tile([13, B_CORE], F32R, tag="xT")
            nc.sync.dma_start(xT[:], xT_d[:].bitcast(F32R))
            Wt = consts.tile([13, len(BD_STEPS) * DIM], F32R, tag="W")
            nc.sync.dma_start(Wt[:], W_d[:].bitcast(F32R))
            Mt = consts.tile([128, 128], F32R, tag="M")
            nc.sync.dma_start(Mt[:], M_d[:].bitcast(F32R))
            Zt = consts.tile([128, 16], F32R, tag="Z")
            nc.sync.dma_start(Zt[:], Z_d[:].bitcast(F32R))
            cst = consts.tile([128, 4 * len(BI_STEPS)], F32, tag="cs")
            nc.sync.dma_start(cst[:], CS_d[:])

            def q_matmuls(k):
                """q = Phi_k/2pi for all 256 states -> psum [128, W2]."""
                qp = psum_pool.tile([128, W2], F32, tag="ps")
                base = BD_POS[k] * DIM
                for h in (0, 1):
                    for c in range(B_CORE // 512):
                        nc.tensor.matmul(
                            qp[:, h * B_CORE + c * 512:
                               h * B_CORE + (c + 1) * 512],
                            Wt[:, base + h * 128: base + (h + 1) * 128],
                            xT[:, c * 512:(c + 1) * 512],
                            start=True, stop=True)
                return qp

            def sincos(qp, SC):
                """SC = [sin(2pi q) | cos(2pi q)] (f16) from q in psum:
                2 rounds + 2 range-reductions + ONE Sin over [fs | fc]."""
                qsi = iq_pool.tile([128, W2], I16, tag="iq")
                nc.vector.tensor_scalar_add(qsi[:], qp[:], 0.0)
                fsc = f_pool.tile([128, 2 * W2], F32, tag="f")
                nc.vector.scalar_tensor_tensor(
                    fsc[:, 0:W2], qp[:], 0.0, qsi[:], AOT.add, AOT.subtract)
                qci = iq_pool.tile([128, W2], I16, tag="iq")
                nc.vector.tensor_scalar_add(qci[:], qp[:], 0.25)
                nc.vector.scalar_tensor_tensor(
                    fsc[:, W2:2 * W2], qp[:], 0.25, qci[:],
                    AOT.add, AOT.subtract)
                nc.scalar.activation(SC[:], fsc[:], ACTF.Sin, scale=TWO_PI)

            def bf_in(st_t):
                """[u | v] = (H_1 (x) 1) st: 2-op butterfly on sbuf."""
                uv = pp_pool.tile([128, W2], F32R, tag="pp")
                nc.vector.tensor_add(uv[:, :B_CORE], st_t[:, :B_CORE],
                                     st_t[:, B_CORE:])
                nc.vector.tensor_sub(uv[:, B_CORE:], st_t[:, :B_CORE],
                                     st_t[:, B_CORE:])
                return uv

            def m_mms(uv):
                """(1 (x) M) uv -> psum [128, W2]."""
                tp = psum_pool.tile([128, W2], F32, tag="ps")
                for c in range(W2 // 512):
                    nc.tensor.matmul(tp[:, c * 512:(c + 1) * 512], Mt[:],
                                     uv[:, c * 512:(c + 1) * 512],
                                     start=True, stop=True)
                return tp

            def h_dmult(st_re, st_im, SC):
                """st' = (C + iS) (*) (H^{x8} (st_re + i st_im)); H = H_1(x)M.
                One mul per psum tile: t broadcast against packed [S|C] gives
                [t*S | t*C]."""
                uvr = bf_in(st_re)
                uvi = bf_in(st_im)
                tre = m_mms(uvr)
                pA = prod_pool.tile([128, 2 * W2], F16, tag="prod")
                nc.vector.tensor_mul(pA[:],
                                     tre[:].unsqueeze(1)
                                     .to_broadcast((128, 2, W2)), SC[:])
                tim = m_mms(uvi)
                pB = prod_pool.tile([128, 2 * W2], F16, tag="prod")
                nc.vector.tensor_mul(pB[:],
                                     tim[:].unsqueeze(1)
                                     .to_broadcast((128, 2, W2)), SC[:])
                n_re = st_pool.tile([128, W2], F32R, tag="st")
                nc.vector.tensor_sub(n_re[:], pA[:, W2:2 * W2], pB[:, 0:W2])
                n_im = st_pool.tile([128, W2], F32R, tag="st")
                nc.vector.tensor_add(n_im[:], pA[:, 0:W2], pB[:, W2:2 * W2])
                return n_re, n_im

            for _rep in range(KREP):
                # init: state = e^{i Phi_0} (x16; H|0> and 1/16 folded into M
                # applications and the final Square scale)
                qp = q_matmuls(0)
                qsi0 = iq_pool.tile([128, W2], I16, tag="iq")
                nc.vector.tensor_scalar_add(qsi0[:], qp[:], 0.0)
                fsc0 = f_pool.tile([128, 2 * W2], F32, tag="f")
                nc.vector.scalar_tensor_tensor(
                    fsc0[:, 0:W2], qp[:], 0.0, qsi0[:], AOT.add, AOT.subtract)
                qci0 = iq_pool.tile([128, W2], I16, tag="iq")
                nc.vector.tensor_scalar_add(qci0[:], qp[:], 0.25)
                nc.vector.scalar_tensor_tensor(
                    fsc0[:, W2:2 * W2], qp[:], 0.25, qci0[:],
                    AOT.add, AOT.subtract)
                st_im = st_pool.tile([128, W2], F32R, tag="st")
                nc.scalar.activation(st_im[:], fsc0[:, 0:W2], ACTF.Sin,
                                     scale=TWO_PI)
                st_re = st_pool.tile([128, W2], F32R, tag="st")
                nc.scalar.activation(st_re[:], fsc0[:, W2:2 * W2], ACTF.Sin,
                                     scale=TWO_PI)

                for k in range(1, NSTEP):
                    SC = sc_pool.tile([128, 2 * W2], F16, tag="sc")
                    if k in BI_STEPS:
                        j = BI_STEPS.index(k)
                        for h in (0, 1):
                            bsl = slice(h * B_CORE, (h + 1) * B_CORE)
                            bsc = slice(W2 + h * B_CORE,
                                        W2 + (h + 1) * B_CORE)
                            nc.vector.tensor_copy(
                                SC[:, bsl],
                                cst[:, 4 * j + 2 + h:4 * j + 3 + h]
                                .to_broadcast((128, B_CORE)))
                            nc.vector.tensor_copy(
                                SC[:, bsc],
                                cst[:, 4 * j + h:4 * j + h + 1]
                                .to_broadcast((128, B_CORE)))
                    else:
                        qp = q_matmuls(k)
                        sincos(qp, SC)
                    st_re, st_im = h_dmult(st_re, st_im, SC)

                # final H, probs, Z-projection
                uvr = bf_in(st_re)
                uvi = bf_in(st_im)
                tre = m_mms(uvr)
                sq1 = sc_pool.tile([128, W2], F32, tag="sc")
                nc.scalar.activation(sq1[:], tre[:], ACTF.Square,
                                     scale=1.0 / 16.0)
                tim = m_mms(uvi)
                sq2 = sc_pool.tile([128, W2], F32, tag="sc")
                nc.scalar.activation(sq2[:], tim[:], ACTF.Square,
                                     scale=1.0 / 16.0)
                probs = pp_pool.tile([128, W2], F32R, tag="pp")
                nc.vector.tensor_add(probs[:], sq1[:], sq2[:])
                zp = psum_pool.tile([128, W2], F32, tag="ps")
                for c in range(B_CORE // 512):
                    csl = slice(c * 512, (c + 1) * 512)
                    nc.tensor.matmul(zp[0:8, csl], Zt[:, 0:8],
                                     probs[:, c * 512:(c + 1) * 512],
                                     start=True, stop=False)
                    nc.tensor.matmul(zp[0:8, csl], Zt[:, 8:16],
                                     probs[:, B_CORE + c * 512:
                                           B_CORE + (c + 1) * 512],
                                     start=False, stop=True)
                zs = tail_pool.tile([8, B_CORE], F32, tag="zs")
                nc.scalar.activation(zs[:], zp[0:8, 0:B_CORE], ACTF.Copy)
                nc.sync.dma_start(out_d[:], zs[:])
    _legalize_waits(nc, limit=int(os.environ.get('LW', '1')),
                    limit_other=int(os.environ.get('LWO', '1')))
    pad = int(os.environ.get('NOOP_PAD', '0'))
    if pad:
        blk = nc.m.functions[0].blocks[1]
        eng = blk.instructions[len(blk.instructions) // 2].engine
        for j in range(pad):
            blk.instructions.insert(
                len(blk.instructions) // 2,
                mybir.InstNoOp(name=f'pad-{j}', engine=eng,
                               sync_info=mybir.SyncInfo(
                                   on_wait=[], on_update=[])))
    return nc


_PROGRAM_CACHE = {}


def make_in_maps(observation, theta, omega):
    observation = np.asarray(observation, np.float32)
    W, CS, M, Zp = _build_host_tables(theta, omega)
    x_augT = np.concatenate(
        [observation, np.ones((B_TOTAL, 1), np.float32)], 1).T  # (13, 16384)

    in_maps = []
    for c in range(N_CORES):
        in_maps.append({
            "xT": np.ascontiguousarray(x_augT[:, c * B_CORE:(c + 1) * B_CORE]),
            "W": W,
            "M": M,
            "Zt": Zp,
            "CS": CS,
        })
    return in_maps


def finalize_output(per_core_out):
    return np.ascontiguousarray(per_core_out.T)


def kernel(observation, theta, omega, _trace=False):
    in_maps = make_in_maps(observation, theta, omega)

    if "nc" not in _PROGRAM_CACHE:
        _PROGRAM_CACHE["nc"] = _build_program()
    nc = _PROGRAM_CACHE["nc"]

    res = run_bass_kernel_spmd(nc, in_maps, core_ids=list(range(N_CORES)),
                               trace=_trace)
    out = np.concatenate([finalize_output(r["out"]) for r in res.results], 0)
    if _trace:
        kernel.last_results = res
    return out

